# revision 31
# baseline (speedup 1.0000x reference)
"""Trainium2 Bass kernel for a 3-layer GCN + mean-pool + MLP head (ModelGraphCoordinationNet).

Strategy (8 NeuronCores, SPMD):
  - Graphs (and their contiguous node ranges) are partitioned across 8 cores,
    balanced by node count. Nodes are re-packed per core into 52 windows of
    128 "slots"; each window holds <= 128 nodes and <= 896 deduplicated
    incoming random edges (7 edge tiles of 128) plus one self-loop tile.
  - GCN conv is computed as aggregate-then-matmul (linearity):
        x_out = ELU(dinv * (sum_e x~[src]) @ W + b),   x~ = dinv * x_in
    The edge aggregation runs on the tensor engine with one-hot "mask"
    matmuls (mask[edge_lane, dst_slot] = edge multiplicity), giving a
    feature-major aggregate that feeds the dense matmul with zero transposes.
  - Layer 1 never materializes node features: W1 is folded into the embedding
    tables host-side (TW[code] = emb[code] @ W1-rows), and per-edge one-hot
    code rows (dinv-premultiplied) are mask-matmul'ed into a count matrix
    that multiplies TW directly.
  - Layers 2/3 gather neighbor rows from an AllGather'd bf16 node-feature
    table with BATCHED multi-column indirect DMAs (4 windows = 28 tiles =
    3584 rows per gpsimd instruction, amortizing the ~1us SWDGE fixed cost).
  - The AllGathers after layers 1/2 are split into 4 chunks of 13 windows,
    fired as soon as their windows are done, overlapping collective time
    with compute. The node-row table uses a chunk-major layout:
        row(core k, window w, slot s) = (w//13)*13312 + k*1664 + (w%13)*128 + s
  - Pooling and the dense head run feature-major per core; host assembles the
    [2000, 1] output from per-core [1, 256] results.
"""
import os
import sys

sys.path.insert(0, '/opt/trn_rl_repo')

import numpy as np
import ml_dtypes

from concourse import bass, bacc, tile, mybir

bf16 = ml_dtypes.bfloat16
f32 = np.float32

# ---------------- static config ----------------
N_NODES = 50000
N_EDGES = 360000
N_GRAPHS = 2000
FDIM = 222
NC = 8
W = 52                 # windows per core
TW = 7                 # random-edge tiles per window
NT = W * (TW + 1)      # tiles per core incl self tiles (416)
NSH = W * 128          # padded rows per core (6656)
ROWS = NC * NSH        # 53248
GSH = 256              # padded graphs per core
EPW = TW * 128         # 896 edge slots per window
XCOL = 256             # padded feature columns
CODES = 200            # one-hot code columns (el 118 | geo 64 | ox 16 | ang 2)
# AllGather chunking: layer-1 output table uses 2 big chunks (CC bw is much
# better on large transfers); layer-2 output uses 4 chunks that overlap the
# long gather phase. Each table has its own chunk-major row layout.
CHUNKS1 = [26, 26]
CHUNKS2 = [39, 13]
AG_LAG2 = 3            # fire layer-2 chunk AGs this many windows late

LAST_EXEC_NS = None    # set after a profiled run
LAST_TRACE_PATH = None

BF = mybir.dt.bfloat16
F32 = mybir.dt.float32
I32 = mybir.dt.int32
AF = mybir.ActivationFunctionType
OP = mybir.AluOpType


# ---------------- host-side preprocessing ----------------

def _prepare(elements, oxidations, geometries, angles, edge_index, batch):
    els = np.asarray(elements).astype(np.int64)
    oxs = np.asarray(oxidations).astype(np.int64)
    geo = np.asarray(geometries).astype(np.int64)
    ang = np.asarray(angles).astype(f32)
    ei = np.asarray(edge_index).astype(np.int64)
    bat = np.asarray(batch).astype(np.int64)

    src, dst = ei[0], ei[1]
    deg_all = np.bincount(dst, minlength=N_NODES) + 1
    dinv_all = (1.0 / np.sqrt(np.maximum(deg_all, 1.0))).astype(f32)

    counts = np.bincount(bat, minlength=N_GRAPHS)
    cum = np.cumsum(counts)
    gb = [0] + [int(np.searchsorted(cum, k * N_NODES / NC)) for k in range(1, NC)] + [N_GRAPHS]
    gb = np.array(gb)
    node_bounds = np.concatenate([[0], cum[gb[1:] - 1]])
    core_of_node = np.searchsorted(node_bounds[1:], np.arange(N_NODES), side='right')

    pair = src * np.int64(N_NODES) + dst
    uniq, cnts = np.unique(pair, return_counts=True)
    usrc = (uniq // N_NODES).astype(np.int64)
    udst = (uniq % N_NODES).astype(np.int64)
    selfm = usrc == udst
    rs, rd, rc = usrc[~selfm], udst[~selfm], cnts[~selfm]
    self_cnt = np.ones(N_NODES, np.int64)
    self_cnt[usrc[selfm]] += cnts[selfm]

    deg_r = np.bincount(rd, minlength=N_NODES)

    # window packing (caps: 128 nodes, EPW edges per window)
    win_of = np.zeros(N_NODES, np.int32)
    slot_of = np.zeros(N_NODES, np.int32)
    for k in range(NC):
        n0, n1 = node_bounds[k], node_bounds[k + 1]
        w = niw = eiw = 0
        dseg = deg_r[n0:n1]
        for i in range(n1 - n0):
            d = int(dseg[i])
            if niw + 1 > 128 or eiw + d > EPW:
                w += 1
                niw = 0
                eiw = 0
            assert w < W, f"window overflow core {k}"
            win_of[n0 + i] = w
            slot_of[n0 + i] = niw
            niw += 1
            eiw += d
    # chunk-major global row layouts (one per AllGather chunking scheme)
    def _layout(chunks):
        starts = np.cumsum([0] + chunks[:-1])
        bases = np.cumsum([0] + [n * NC * 128 for n in chunks[:-1]])
        cof = np.searchsorted(np.cumsum(chunks), win_of, side='right')
        return (bases[cof] + core_of_node * (np.array(chunks)[cof] * 128)
                + (win_of - starts[cof]) * 128 + slot_of)

    padded_row1 = _layout(CHUNKS1)
    padded_row2 = _layout(CHUNKS2)

    cores = []
    for k in range(NC):
        n0, n1 = node_bounds[k], node_bounds[k + 1]
        em = core_of_node[rd] == k
        es, ed, ec = rs[em], rd[em], rc[em]
        ew = win_of[ed]
        order = np.argsort(ew, kind='stable')
        es, ed, ec, ew = es[order], ed[order], ec[order], ew[order]

        gidx2 = np.zeros((128, W * TW), np.int32)
        gidx3 = np.zeros((128, W * TW), np.int32)
        mask = np.zeros((128, NT * 128), f32)
        src_node = np.full((128, NT), -1, np.int64)

        # per-window positions via cumulative counts (vectorized fill)
        wstart = np.searchsorted(ew, np.arange(W))
        pos = np.arange(len(es)) - wstart[ew]
        assert (pos < EPW).all(), f"edge overflow core {k}"
        t = pos // 128
        lane = pos % 128
        tl = ew * (TW + 1) + t
        gidx2[lane, ew * TW + t] = padded_row1[es]
        gidx3[lane, ew * TW + t] = padded_row2[es]
        mask[lane, tl * 128 + slot_of[ed]] = ec
        src_node[lane, tl] = es

        nodes = np.arange(n0, n1)
        wn, sn = win_of[nodes], slot_of[nodes]
        tself = wn * (TW + 1) + TW
        mask[sn, tself * 128 + sn] = self_cnt[nodes]
        src_node[sn, tself] = nodes

        # per-edge one-hot code rows, premultiplied by dinv[src]
        ohr = np.zeros((128, NT, CODES), f32)
        valid = src_node >= 0
        sv = src_node[valid]
        dv = dinv_all[sv]
        lane_i, tile_i = np.nonzero(valid)
        ohr[lane_i, tile_i, els[sv]] = dv
        ohr[lane_i, tile_i, 118 + geo[sv]] = dv
        ohr[lane_i, tile_i, 182 + oxs[sv]] = dv
        ohr[lane_i, tile_i, 198] = ang[sv, 0] * dv
        ohr[lane_i, tile_i, 199] = ang[sv, 1] * dv

        deg_node = np.ones((128, W), f32)
        deg_node[sn, wn] = deg_all[nodes]

        pm = np.zeros((128, W * GSH), f32)
        lg = bat[nodes] - gb[k]
        pm[sn, wn * GSH + lg] = 1.0

        gcnt = np.bincount(lg, minlength=GSH).astype(f32)
        cinv = (1.0 / np.maximum(gcnt, 1.0)).reshape(1, GSH)

        cores.append(dict(
            gidx2=gidx2, gidx3=gidx3, mask=mask.astype(ml_dtypes.float8_e4m3),
            ohr=np.ascontiguousarray(ohr.reshape(128, NT * CODES)).astype(bf16),
            deg_node=deg_node.astype(bf16), pm=pm.astype(bf16), cinv=cinv,
            n_graphs=int(gb[k + 1] - gb[k]),
        ))
    return dict(graph_bounds=gb, cores=cores)


def _pack_weights(inp):
    d = {}
    # layer 1 folded tables: TW[code, :] rows in code layout
    # [el 0:118 | geo 118:182 | ox 182:198 | ang 198:200]
    W1 = np.asarray(inp['W1'], f32)
    TWt = np.zeros((CODES, FDIM), f32)
    TWt[0:118] = np.asarray(inp['emb_element'], f32) @ W1[0:200]
    TWt[118:182] = np.asarray(inp['emb_geo'], f32) @ W1[210:220]
    TWt[182:198] = np.asarray(inp['emb_ox'], f32) @ W1[200:210]
    TWt[198:200] = W1[220:222]
    d['TWa'] = np.ascontiguousarray(TWt[0:128]).astype(bf16)
    d['TWb'] = np.ascontiguousarray(TWt[128:200]).astype(bf16)
    for l in (1, 2, 3):
        if l > 1:
            Wl = np.asarray(inp[f'W{l}'], f32)
            d[f'W{l}a'] = np.ascontiguousarray(Wl[0:128, :]).astype(bf16)
            d[f'W{l}b'] = np.ascontiguousarray(Wl[128:222, :]).astype(bf16)
        d[f'bias{l}'] = np.broadcast_to(np.asarray(inp[f'b{l}'], f32), (128, FDIM)).copy()
    Wd1 = np.asarray(inp['Wd1'], f32)
    d['Wd1a'] = np.ascontiguousarray(Wd1[0:128, :]).astype(bf16)
    d['Wd1b'] = np.ascontiguousarray(Wd1[128:222, :]).astype(bf16)
    Wd2 = np.asarray(inp['Wd2'], f32)
    d['Wd2p'] = np.concatenate([Wd2[128 * m:128 * (m + 1), :] for m in range(4)], axis=1).astype(bf16)
    d['Wd3p'] = np.asarray(inp['Wd3'], f32).astype(bf16)
    d['bd1p'] = np.ascontiguousarray(np.asarray(inp['bd1'], f32).reshape(4, 128).T)
    d['bd2p'] = np.asarray(inp['bd2'], f32).reshape(128, 1).copy()
    d['bd3p'] = np.asarray(inp['bd3'], f32).reshape(1, 1).copy()

    d['ones_row'] = np.ones((1, 128), f32)
    return d


# ---------------- bass kernel ----------------

_PER_CORE_SPECS = [
    ('gidx2', [128, W * TW], I32),
    ('gidx3', [128, W * TW], I32),
    ('mask', [128, NT * 128], mybir.dt.float8e4),
    ('ohr', [128, NT * CODES], BF),
    ('deg_node', [128, W], BF), ('pm', [128, W * GSH], BF),
    ('cinv', [1, GSH], F32),
]
_SHARED_SPECS = [
    ('TWa', [128, FDIM], BF), ('TWb', [72, FDIM], BF),
    ('bias1', [128, FDIM], F32),
    ('W2a', [128, FDIM], BF), ('W2b', [94, FDIM], BF), ('bias2', [128, FDIM], F32),
    ('W3a', [128, FDIM], BF), ('W3b', [94, FDIM], BF), ('bias3', [128, FDIM], F32),
    ('Wd1a', [128, 512], BF), ('Wd1b', [94, 512], BF),
    ('Wd2p', [128, 512], BF), ('Wd3p', [128, 1], BF),
    ('bd1p', [128, 4], F32), ('bd2p', [128, 1], F32), ('bd3p', [1, 1], F32),
    ('ones_row', [1, 128], F32),
]

_BUILT = None


def _build():
    global _BUILT
    if _BUILT is not None:
        return _BUILT

    nc = bacc.Bacc("TRN2", target_bir_lowering=False, debug=False,
                   enable_asserts=False, num_devices=NC)

    dram_in = {}
    for name, shape, dt in _PER_CORE_SPECS + _SHARED_SPECS:
        dram_in[name] = nc.dram_tensor(name, shape, dt, kind="ExternalInput")
    out_t = nc.dram_tensor("out", [1, GSH], F32, kind="ExternalOutput")

    with tile.TileContext(nc) as tc:
        with tc.tile_pool(name="res", bufs=1) as res, \
             tc.tile_pool(name="dram", bufs=1, space="DRAM") as dram, \
             tc.tile_pool(name="wrk", bufs=2) as wrk, \
             tc.tile_pool(name="feats", bufs=6) as fpool, \
             tc.tile_pool(name="oh", bufs=4) as ohpool, \
             tc.tile_pool(name="post", bufs=2) as post, \
             tc.tile_pool(name="aggs", bufs=6) as aggs:

            # ---- resident tiles ----
            sb = {}
            for name, shape, dt in _PER_CORE_SPECS + _SHARED_SPECS:
                if name in ('pm', 'ohr'):
                    continue        # streamed
                t_ = res.tile(shape, dt, tag=name, name=f'sb_{name}')
                nc.sync.dma_start(t_[:], dram_in[name].ap())
                sb[name] = t_

            arenaA = res.tile([128, W, XCOL], BF, tag="arenaA")
            arenaB = res.tile([128, W, XCOL], BF, tag="arenaB")
            nc.vector.memset(arenaA[:, :, FDIM:XCOL], 0.0)
            nc.scalar.memzero(arenaB[:, :, FDIM:XCOL])

            # dinv per node [128, W] f32
            dinv = res.tile([128, W], F32, tag="dinv")
            tmp = wrk.tile([128, W], F32, tag="dtmp")
            nc.vector.tensor_scalar_max(tmp[:], sb['deg_node'][:], 1.0)
            nc.scalar.sqrt(tmp[:], tmp[:])
            nc.vector.reciprocal(dinv[:], tmp[:])

            cc_in = {}
            cc_out = {}
            chunks_of = {1: CHUNKS1, 2: CHUNKS2}
            for l in (1, 2):
                cc_in[l] = [dram.tile([n * 128, XCOL], BF, tag=f"ccin{l}_{c}",
                                      name=f"ccin{l}_{c}")
                            for c, n in enumerate(chunks_of[l])]
                cc_out[l] = dram.tile([ROWS, XCOL], BF,
                                      tag=f"ccout{l}", name=f"ccout{l}")

            def mask_tile(w, t):
                tl = w * (TW + 1) + t
                return sb['mask'][:, tl * 128:(tl + 1) * 128]

            def post_ops(w, hP, l, arena):
                """arena[:, w, 0:FDIM] = [dinv *] ELU(dinv * h + b)."""
                last = l == 3
                bias = sb[f'bias{l}']
                u = post.tile([128, FDIM], F32, tag="u")
                nc.vector.scalar_tensor_tensor(u[:], hP[:], dinv[:, w:w + 1], bias[:],
                                               op0=OP.mult, op1=OP.add)
                v = post.tile([128, FDIM], F32, tag="v")
                nc.vector.tensor_scalar_min(v[:], u[:], 0.0)
                e = post.tile([128, FDIM], F32, tag="e")
                nc.scalar.activation(e[:], v[:], AF.Exp)
                r = post.tile([128, FDIM], F32, tag="r")
                nc.scalar.activation(r[:], u[:], AF.Relu)
                dst = arena[:, w, 0:FDIM]
                if last:
                    nc.vector.scalar_tensor_tensor(dst, e[:], -1.0, r[:],
                                                   op0=OP.add, op1=OP.add)
                else:
                    t2 = post.tile([128, FDIM], F32, tag="t2")
                    nc.vector.scalar_tensor_tensor(t2[:], e[:], -1.0, r[:],
                                                   op0=OP.add, op1=OP.add)
                    nc.scalar.activation(dst, t2[:], AF.Copy, scale=dinv[:, w:w + 1])

            # Per-layer chunk metadata: window -> chunk, chunk row offsets, and
            # the window at which each chunk's AllGather fires (lagged for l=2
            # so the collective's wait doesn't stall the gpsimd gather stream).
            _chunk_meta = {}
            for l in (1, 2):
                chs = chunks_of[l]
                starts = np.cumsum([0] + chs[:-1])
                bases = np.cumsum([0] + [n * NC * 128 for n in chs[:-1]])
                lag = AG_LAG2 if l == 2 else 0
                fire = {}
                for c, n in enumerate(chs):
                    fw = min(starts[c] + n - 1 + (lag if c < len(chs) - 1 else 0), W - 1)
                    fire[fw] = c
                _chunk_meta[l] = (list(starts), list(bases), chs, fire)

            def emit_store_and_ag(l, w, arena):
                """DMA window w rows to its cc_in chunk; fire AG at chunk end."""
                starts, bases, chs, fire = _chunk_meta[l]
                c = int(np.searchsorted(np.cumsum(chs), w, side='right'))
                r0 = (w - starts[c]) * 128
                nc.sync.dma_start(cc_in[l][c][r0:r0 + 128, :], arena[:, w, :])
                if w in fire:
                    cf = fire[w]
                    out0 = bases[cf]
                    outn = chs[cf] * NC * 128
                    nc.gpsimd.collective_compute(
                        "AllGather", OP.bypass, replica_groups=[list(range(NC))],
                        ins=[cc_in[l][cf][:]],
                        outs=[cc_out[l][out0:out0 + outn, :]])

            ctx_pH = tc.tile_pool(name="pH", bufs=2, space="PSUM")
            pH = ctx_pH.__enter__()

            # ================= layer 1 (folded-W1 C-matrix path) =================
            pL1 = ctx_pL1 = tc.tile_pool(name="pL1", bufs=2, space="PSUM")
            pL1 = ctx_pL1.__enter__()
            mask3 = sb['mask'][:].rearrange("p (t c) -> p t c", c=128)
            ohr3 = dram_in['ohr'].ap().rearrange("p (t c) -> p t c", c=CODES)
            # Software-pipelined by one window: dense(w-1) is emitted after
            # cP(w)'s matmuls so the PE never stalls on the A1/A2 evictions.
            prevA = None
            for w in range(W + 1):
                if w < W:
                    t0 = w * (TW + 1)
                    oh = ohpool.tile([128, TW + 1, CODES], BF, tag="oh")
                    nc.sync.dma_start(oh[:], ohr3[:, t0:t0 + TW + 1, :])
                    cP1 = pL1.tile([128, 128], F32, tag="cP1", name="cP1")
                    cP2 = pL1.tile([72, 128], F32, tag="cP2", name="cP2")
                    for t in range(TW + 1):
                        nc.tensor.matmul(cP1[:], lhsT=oh[:, t, 0:128], rhs=mask3[:, t0 + t, :],
                                         start=(t == 0), stop=(t == TW))
                        nc.tensor.matmul(cP2[:], lhsT=oh[:, t, 128:CODES], rhs=mask3[:, t0 + t, :],
                                         start=(t == 0), stop=(t == TW))
                if prevA is not None:
                    A1p, A2p, wp = prevA
                    hP = pH.tile([128, FDIM], F32, tag="h", name="hP1")
                    nc.tensor.matmul(hP[:], lhsT=A1p[:], rhs=sb['TWa'][:], start=True, stop=False)
                    nc.tensor.matmul(hP[:], lhsT=A2p[:], rhs=sb['TWb'][:], start=False, stop=True)
                    post_ops(wp, hP, 1, arenaA)
                    emit_store_and_ag(1, wp, arenaA)
                if w < W:
                    A1 = aggs.tile([128, 128], BF, tag="A1")
                    nc.vector.tensor_copy(A1[:], cP1[:])
                    A2 = aggs.tile([72, 128], BF, tag="A2x", name="A2x")
                    nc.scalar.copy(A2[:], cP2[:])
                    prevA = (A1, A2, w)

            ctx_pL1.__exit__(None, None, None)
            ctx_pA = tc.tile_pool(name="pA", bufs=2, space="PSUM")
            pA = ctx_pA.__enter__()

            # ================= layers 2 and 3 =================
            g1 = pA.tile([128, GSH], F32, tag="g1", name="g1", bufs=1)
            g2 = pA.tile([94, GSH], F32, tag="g2", name="g2", bufs=1)
            for l, arena_prev, arena_next, gname in ((2, arenaA, arenaB, 'gidx2'),
                                                     (3, arenaB, arenaA, 'gidx3')):
                src_full = cc_out[l - 1]
                for w in range(W):
                    aggP1 = pA.tile([128, 128], F32, tag="agg1")
                    aggP2 = pA.tile([94, 128], F32, tag="agg2")
                    fW = fpool.tile([128, TW, XCOL], BF, tag="F")
                    for t in range(TW):
                        nc.gpsimd.indirect_dma_start(
                            out=fW[:, t, :], out_offset=None, in_=src_full[:],
                            in_offset=bass.IndirectOffsetOnAxis(
                                ap=sb[gname][:, w * TW + t:w * TW + t + 1], axis=0))
                    for t in range(TW):
                        fT = fW[:, t, :]
                        nc.tensor.matmul(aggP1[:], lhsT=fT[:, 0:128], rhs=mask_tile(w, t),
                                         start=(t == 0), stop=False)
                        nc.tensor.matmul(aggP2[:], lhsT=fT[:, 128:FDIM], rhs=mask_tile(w, t),
                                         start=(t == 0), stop=False)
                    nc.tensor.matmul(aggP1[:], lhsT=arena_prev[:, w, 0:128],
                                     rhs=mask_tile(w, TW), start=False, stop=True)
                    nc.tensor.matmul(aggP2[:], lhsT=arena_prev[:, w, 128:FDIM],
                                     rhs=mask_tile(w, TW), start=False, stop=True)
                    A1 = aggs.tile([128, 128], BF, tag="A1")
                    nc.vector.tensor_copy(A1[:], aggP1[:])
                    A2 = aggs.tile([94, 128], BF, tag="A2")
                    nc.scalar.copy(A2[:], aggP2[:])
                    hP = pH.tile([128, FDIM], F32, tag="h")
                    nc.tensor.matmul(hP[:], lhsT=A1[:], rhs=sb[f'W{l}a'][:], start=True, stop=False)
                    nc.tensor.matmul(hP[:], lhsT=A2[:], rhs=sb[f'W{l}b'][:], start=False, stop=True)
                    post_ops(w, hP, l, arena_next)
                    if l == 2:
                        emit_store_and_ag(2, w, arena_next)
                    if l == 3:
                        pmT = wrk.tile([128, GSH], BF, tag="pm")
                        nc.sync.dma_start(pmT[:], dram_in['pm'].ap()[:, w * GSH:(w + 1) * GSH])
                        nc.tensor.matmul(g1[:], lhsT=arena_next[:, w, 0:128], rhs=pmT[:],
                                         start=(w == 0), stop=(w == W - 1))
                        nc.tensor.matmul(g2[:], lhsT=arena_next[:, w, 128:FDIM], rhs=pmT[:],
                                         start=(w == 0), stop=(w == W - 1))

            # evict pooled sums, then free psum pools for the head
            g1s = res.tile([128, GSH], F32, tag="g1s")
            nc.scalar.copy(g1s[:], g1[:])
            g2s = res.tile([94, GSH], F32, tag="g2s")
            nc.scalar.copy(g2s[:], g2[:])
            ctx_pA.__exit__(None, None, None)
            ctx_pH.__exit__(None, None, None)

            # ================= head =================
            ctx_pPH = tc.tile_pool(name="pPH", bufs=1, space="PSUM")
            pPH = ctx_pPH.__enter__()
            cibP = pPH.tile([128, GSH], F32, tag="cib", name="cibP")
            nc.tensor.matmul(cibP[:], lhsT=sb['ones_row'][:], rhs=sb['cinv'][:],
                             start=True, stop=True)
            cib = wrk.tile([128, GSH], F32, tag="cibs")
            nc.scalar.copy(cib[:], cibP[:])
            gs1 = res.tile([128, GSH], BF, tag="gs1")
            nc.vector.tensor_tensor(gs1[:], g1s[:], cib[:], op=OP.mult)
            gs2 = res.tile([94, GSH], BF, tag="gs2")
            nc.vector.tensor_tensor(gs2[:], g2s[:], cib[0:94, :], op=OP.mult)

            def elu_head(hp, bias_ap, out_bf):
                u = post.tile(out_bf.shape, F32, tag="u")
                nc.vector.tensor_scalar(u[:], hp[:], bias_ap, None, op0=OP.add)
                v = post.tile(out_bf.shape, F32, tag="v")
                nc.vector.tensor_scalar_min(v[:], u[:], 0.0)
                e = post.tile(out_bf.shape, F32, tag="e")
                nc.scalar.activation(e[:], v[:], AF.Exp)
                r = post.tile(out_bf.shape, F32, tag="r")
                nc.scalar.activation(r[:], u[:], AF.Relu)
                nc.vector.scalar_tensor_tensor(out_bf[:], e[:], -1.0, r[:],
                                               op0=OP.add, op1=OP.add)

            hs1 = []
            for m in range(4):
                hp = pPH.tile([128, GSH], F32, tag="h1p", bufs=2, name="hp")
                nc.tensor.matmul(hp[:], lhsT=sb['Wd1a'][:, 128 * m:128 * (m + 1)],
                                 rhs=gs1[:], start=True, stop=False)
                nc.tensor.matmul(hp[:], lhsT=sb['Wd1b'][:, 128 * m:128 * (m + 1)],
                                 rhs=gs2[:], start=False, stop=True)
                hb = res.tile([128, GSH], BF, tag=f"hs1_{m}")
                elu_head(hp, sb['bd1p'][:, m:m + 1], hb)
                hs1.append(hb)
            h2p = pPH.tile([128, GSH], F32, tag="h2p", name="h2p")
            for m in range(4):
                nc.tensor.matmul(h2p[:], lhsT=sb['Wd2p'][:, 128 * m:128 * (m + 1)],
                                 rhs=hs1[m][:], start=(m == 0), stop=(m == 3))
            hs2 = res.tile([128, GSH], BF, tag="hs2")
            elu_head(h2p, sb['bd2p'][:, 0:1], hs2)
            op_ = pPH.tile([1, GSH], F32, tag="outp", name="op_")
            nc.tensor.matmul(op_[:], lhsT=sb['Wd3p'][:], rhs=hs2[:], start=True, stop=True)
            outS = wrk.tile([1, GSH], F32, tag="outS")
            nc.vector.tensor_scalar(outS[:], op_[:], sb['bd3p'][0:1, 0:1], None, op0=OP.add)
            nc.sync.dma_start(out_t.ap(), outS[:])
            ctx_pPH.__exit__(None, None, None)

    nc.compile()
    _BUILT = (nc, out_t.name)
    return _BUILT


# ---------------- public entry point ----------------

def kernel(elements, oxidations, geometries, angles, edge_index, batch,
           emb_element, emb_ox, emb_geo,
           W1, b1, W2, b2, W3, b3,
           Wd1, bd1, Wd2, bd2, Wd3, bd3):
    global LAST_EXEC_NS, LAST_TRACE_PATH
    inp = dict(elements=elements, oxidations=oxidations, geometries=geometries,
               angles=angles, edge_index=edge_index, batch=batch,
               emb_element=emb_element, emb_ox=emb_ox, emb_geo=emb_geo,
               W1=W1, b1=b1, W2=W2, b2=b2, W3=W3, b3=b3,
               Wd1=Wd1, bd1=bd1, Wd2=Wd2, bd2=bd2, Wd3=Wd3, bd3=bd3)
    pp = _prepare(elements, oxidations, geometries, angles, edge_index, batch)
    wts = _pack_weights(inp)
    nc, out_name = _build()

    in_maps = []
    for k in range(NC):
        c = pp['cores'][k]
        m = {name: c[name] for name, _, _ in _PER_CORE_SPECS}
        for name, _, _ in _SHARED_SPECS:
            m[name] = wts[name]
        in_maps.append(m)

    from concourse import bass_utils
    trace = bool(int(os.environ.get('KERNEL_PROFILE', '0')))
    res = bass_utils.run_bass_kernel_spmd(nc, in_maps, core_ids=list(range(NC)),
                                          trace=trace)
    LAST_EXEC_NS = res.exec_time_ns
    if res.instructions_and_trace is not None:
        LAST_TRACE_PATH = res.instructions_and_trace[1]

    gb = pp['graph_bounds']
    out = np.zeros((N_GRAPHS, 1), f32)
    for k in range(NC):
        ng = pp['cores'][k]['n_graphs']
        out[gb[k]:gb[k + 1], 0] = res.results[k][out_name][0, :ng]
    return out


# revision 33
# speedup vs baseline: 1.0208x; 1.0208x over previous
"""Trainium2 Bass kernel for a 3-layer GCN + mean-pool + MLP head (ModelGraphCoordinationNet).

Strategy (8 NeuronCores, SPMD):
  - Graphs (and their contiguous node ranges) are partitioned across 8 cores,
    balanced by node count. Nodes are re-packed per core into 52 windows of
    128 "slots"; each window holds <= 128 nodes and <= 896 deduplicated
    incoming random edges (7 edge tiles of 128) plus one self-loop tile.
  - GCN conv is computed as aggregate-then-matmul (linearity):
        x_out = ELU(dinv * (sum_e x~[src]) @ W + b),   x~ = dinv * x_in
    The edge aggregation runs on the tensor engine with one-hot "mask"
    matmuls (mask[edge_lane, dst_slot] = edge multiplicity), giving a
    feature-major aggregate that feeds the dense matmul with zero transposes.
  - Layer 1 never materializes node features: W1 is folded into the embedding
    tables host-side (TW[code] = emb[code] @ W1-rows), and per-edge one-hot
    code rows (dinv-premultiplied) are mask-matmul'ed into a count matrix
    that multiplies TW directly.
  - Layers 2/3 gather neighbor rows from an AllGather'd bf16 node-feature
    table with BATCHED multi-column indirect DMAs (4 windows = 28 tiles =
    3584 rows per gpsimd instruction, amortizing the ~1us SWDGE fixed cost).
  - The AllGathers after layers 1/2 are split into 4 chunks of 13 windows,
    fired as soon as their windows are done, overlapping collective time
    with compute. The node-row table uses a chunk-major layout:
        row(core k, window w, slot s) = (w//13)*13312 + k*1664 + (w%13)*128 + s
  - Pooling and the dense head run feature-major per core; host assembles the
    [2000, 1] output from per-core [1, 256] results.
"""
import os
import sys

sys.path.insert(0, '/opt/trn_rl_repo')

import numpy as np
import ml_dtypes

from concourse import bass, bacc, tile, mybir

bf16 = ml_dtypes.bfloat16
f32 = np.float32

# ---------------- static config ----------------
N_NODES = 50000
N_EDGES = 360000
N_GRAPHS = 2000
FDIM = 222
NC = 8
W = 52                 # windows per core
TW = 7                 # random-edge tiles per window
NT = W * (TW + 1)      # tiles per core incl self tiles (416)
NSH = W * 128          # padded rows per core (6656)
ROWS = NC * NSH        # 53248
GSH = 256              # padded graphs per core
EPW = TW * 128         # 896 edge slots per window
XCOL = 256             # padded feature columns
CODES = 200            # one-hot code columns (el 118 | geo 64 | ox 16 | ang 2)
# AllGather chunking: layer-1 output table uses 2 big chunks (CC bw is much
# better on large transfers); layer-2 output uses 4 chunks that overlap the
# long gather phase. Each table has its own chunk-major row layout.
CHUNKS1 = [26, 26]
CHUNKS2 = [13, 13, 13, 13]
AG_LAG2 = 3            # fire layer-2 chunk AGs this many windows late

LAST_EXEC_NS = None    # set after a profiled run
LAST_TRACE_PATH = None

BF = mybir.dt.bfloat16
F32 = mybir.dt.float32
I32 = mybir.dt.int32
AF = mybir.ActivationFunctionType
OP = mybir.AluOpType


# ---------------- host-side preprocessing ----------------

def _prepare(elements, oxidations, geometries, angles, edge_index, batch):
    els = np.asarray(elements).astype(np.int64)
    oxs = np.asarray(oxidations).astype(np.int64)
    geo = np.asarray(geometries).astype(np.int64)
    ang = np.asarray(angles).astype(f32)
    ei = np.asarray(edge_index).astype(np.int64)
    bat = np.asarray(batch).astype(np.int64)

    src, dst = ei[0], ei[1]
    deg_all = np.bincount(dst, minlength=N_NODES) + 1
    dinv_all = (1.0 / np.sqrt(np.maximum(deg_all, 1.0))).astype(f32)

    counts = np.bincount(bat, minlength=N_GRAPHS)
    cum = np.cumsum(counts)
    gb = [0] + [int(np.searchsorted(cum, k * N_NODES / NC)) for k in range(1, NC)] + [N_GRAPHS]
    gb = np.array(gb)
    node_bounds = np.concatenate([[0], cum[gb[1:] - 1]])
    core_of_node = np.searchsorted(node_bounds[1:], np.arange(N_NODES), side='right')

    pair = src * np.int64(N_NODES) + dst
    uniq, cnts = np.unique(pair, return_counts=True)
    usrc = (uniq // N_NODES).astype(np.int64)
    udst = (uniq % N_NODES).astype(np.int64)
    selfm = usrc == udst
    rs, rd, rc = usrc[~selfm], udst[~selfm], cnts[~selfm]
    self_cnt = np.ones(N_NODES, np.int64)
    self_cnt[usrc[selfm]] += cnts[selfm]

    deg_r = np.bincount(rd, minlength=N_NODES)

    # window packing (caps: 128 nodes, EPW edges per window)
    win_of = np.zeros(N_NODES, np.int32)
    slot_of = np.zeros(N_NODES, np.int32)
    for k in range(NC):
        n0, n1 = node_bounds[k], node_bounds[k + 1]
        w = niw = eiw = 0
        dseg = deg_r[n0:n1]
        for i in range(n1 - n0):
            d = int(dseg[i])
            if niw + 1 > 128 or eiw + d > EPW:
                w += 1
                niw = 0
                eiw = 0
            assert w < W, f"window overflow core {k}"
            win_of[n0 + i] = w
            slot_of[n0 + i] = niw
            niw += 1
            eiw += d
    # chunk-major global row layouts (one per AllGather chunking scheme)
    def _layout(chunks):
        starts = np.cumsum([0] + chunks[:-1])
        bases = np.cumsum([0] + [n * NC * 128 for n in chunks[:-1]])
        cof = np.searchsorted(np.cumsum(chunks), win_of, side='right')
        return (bases[cof] + core_of_node * (np.array(chunks)[cof] * 128)
                + (win_of - starts[cof]) * 128 + slot_of)

    padded_row1 = _layout(CHUNKS1)
    padded_row2 = _layout(CHUNKS2)

    cores = []
    for k in range(NC):
        n0, n1 = node_bounds[k], node_bounds[k + 1]
        em = core_of_node[rd] == k
        es, ed, ec = rs[em], rd[em], rc[em]
        ew = win_of[ed]
        order = np.argsort(ew, kind='stable')
        es, ed, ec, ew = es[order], ed[order], ec[order], ew[order]

        gidx2 = np.zeros((128, W * TW), np.int32)
        gidx3 = np.zeros((128, W * TW), np.int32)
        mask = np.zeros((128, NT * 128), f32)
        src_node = np.full((128, NT), -1, np.int64)

        # per-window positions via cumulative counts (vectorized fill)
        wstart = np.searchsorted(ew, np.arange(W))
        pos = np.arange(len(es)) - wstart[ew]
        assert (pos < EPW).all(), f"edge overflow core {k}"
        t = pos // 128
        lane = pos % 128
        tl = ew * (TW + 1) + t
        gidx2[lane, ew * TW + t] = padded_row1[es]
        gidx3[lane, ew * TW + t] = padded_row2[es]
        mask[lane, tl * 128 + slot_of[ed]] = ec
        src_node[lane, tl] = es

        nodes = np.arange(n0, n1)
        wn, sn = win_of[nodes], slot_of[nodes]
        tself = wn * (TW + 1) + TW
        mask[sn, tself * 128 + sn] = self_cnt[nodes]
        src_node[sn, tself] = nodes

        # per-edge one-hot code rows, premultiplied by dinv[src]
        ohr = np.zeros((128, NT, CODES), f32)
        valid = src_node >= 0
        sv = src_node[valid]
        dv = dinv_all[sv]
        lane_i, tile_i = np.nonzero(valid)
        ohr[lane_i, tile_i, els[sv]] = dv
        ohr[lane_i, tile_i, 118 + geo[sv]] = dv
        ohr[lane_i, tile_i, 182 + oxs[sv]] = dv
        ohr[lane_i, tile_i, 198] = ang[sv, 0] * dv
        ohr[lane_i, tile_i, 199] = ang[sv, 1] * dv

        deg_node = np.ones((128, W), f32)
        deg_node[sn, wn] = deg_all[nodes]

        pm = np.zeros((128, W * GSH), f32)
        lg = bat[nodes] - gb[k]
        pm[sn, wn * GSH + lg] = 1.0

        gcnt = np.bincount(lg, minlength=GSH).astype(f32)
        cinv = (1.0 / np.maximum(gcnt, 1.0)).reshape(1, GSH)

        cores.append(dict(
            gidx2=gidx2, gidx3=gidx3, mask=mask.astype(ml_dtypes.float8_e4m3),
            ohr=np.ascontiguousarray(ohr.reshape(128, NT * CODES)).astype(bf16),
            deg_node=deg_node.astype(bf16), pm=pm.astype(bf16), cinv=cinv,
            n_graphs=int(gb[k + 1] - gb[k]),
        ))
    return dict(graph_bounds=gb, cores=cores)


def _pack_weights(inp):
    d = {}
    # layer 1 folded tables: TW[code, :] rows in code layout
    # [el 0:118 | geo 118:182 | ox 182:198 | ang 198:200]
    W1 = np.asarray(inp['W1'], f32)
    TWt = np.zeros((CODES, FDIM), f32)
    TWt[0:118] = np.asarray(inp['emb_element'], f32) @ W1[0:200]
    TWt[118:182] = np.asarray(inp['emb_geo'], f32) @ W1[210:220]
    TWt[182:198] = np.asarray(inp['emb_ox'], f32) @ W1[200:210]
    TWt[198:200] = W1[220:222]
    d['TWa'] = np.ascontiguousarray(TWt[0:128]).astype(bf16)
    d['TWb'] = np.ascontiguousarray(TWt[128:200]).astype(bf16)
    for l in (1, 2, 3):
        if l > 1:
            Wl = np.asarray(inp[f'W{l}'], f32)
            d[f'W{l}a'] = np.ascontiguousarray(Wl[0:128, :]).astype(bf16)
            d[f'W{l}b'] = np.ascontiguousarray(Wl[128:222, :]).astype(bf16)
        d[f'bias{l}'] = np.broadcast_to(np.asarray(inp[f'b{l}'], f32), (128, FDIM)).copy()
    Wd1 = np.asarray(inp['Wd1'], f32)
    d['Wd1a'] = np.ascontiguousarray(Wd1[0:128, :]).astype(bf16)
    d['Wd1b'] = np.ascontiguousarray(Wd1[128:222, :]).astype(bf16)
    Wd2 = np.asarray(inp['Wd2'], f32)
    d['Wd2p'] = np.concatenate([Wd2[128 * m:128 * (m + 1), :] for m in range(4)], axis=1).astype(bf16)
    d['Wd3p'] = np.asarray(inp['Wd3'], f32).astype(bf16)
    d['bd1p'] = np.ascontiguousarray(np.asarray(inp['bd1'], f32).reshape(4, 128).T)
    d['bd2p'] = np.asarray(inp['bd2'], f32).reshape(128, 1).copy()
    d['bd3p'] = np.asarray(inp['bd3'], f32).reshape(1, 1).copy()

    d['ones_row'] = np.ones((1, 128), f32)
    return d


# ---------------- bass kernel ----------------

_PER_CORE_SPECS = [
    ('gidx2', [128, W * TW], I32),
    ('gidx3', [128, W * TW], I32),
    ('mask', [128, NT * 128], mybir.dt.float8e4),
    ('ohr', [128, NT * CODES], BF),
    ('deg_node', [128, W], BF), ('pm', [128, W * GSH], BF),
    ('cinv', [1, GSH], F32),
]
_SHARED_SPECS = [
    ('TWa', [128, FDIM], BF), ('TWb', [72, FDIM], BF),
    ('bias1', [128, FDIM], F32),
    ('W2a', [128, FDIM], BF), ('W2b', [94, FDIM], BF), ('bias2', [128, FDIM], F32),
    ('W3a', [128, FDIM], BF), ('W3b', [94, FDIM], BF), ('bias3', [128, FDIM], F32),
    ('Wd1a', [128, 512], BF), ('Wd1b', [94, 512], BF),
    ('Wd2p', [128, 512], BF), ('Wd3p', [128, 1], BF),
    ('bd1p', [128, 4], F32), ('bd2p', [128, 1], F32), ('bd3p', [1, 1], F32),
    ('ones_row', [1, 128], F32),
]

_BUILT = None


def _build():
    global _BUILT
    if _BUILT is not None:
        return _BUILT

    nc = bacc.Bacc("TRN2", target_bir_lowering=False, debug=False,
                   enable_asserts=False, num_devices=NC)

    dram_in = {}
    for name, shape, dt in _PER_CORE_SPECS + _SHARED_SPECS:
        dram_in[name] = nc.dram_tensor(name, shape, dt, kind="ExternalInput")
    out_t = nc.dram_tensor("out", [1, GSH], F32, kind="ExternalOutput")

    with tile.TileContext(nc) as tc:
        with tc.tile_pool(name="res", bufs=1) as res, \
             tc.tile_pool(name="dram", bufs=1, space="DRAM") as dram, \
             tc.tile_pool(name="wrk", bufs=2) as wrk, \
             tc.tile_pool(name="feats", bufs=4) as fpool, \
             tc.tile_pool(name="oh", bufs=4) as ohpool, \
             tc.tile_pool(name="post", bufs=2) as post, \
             tc.tile_pool(name="aggs", bufs=6) as aggs:

            # ---- resident tiles ----
            sb = {}
            for name, shape, dt in _PER_CORE_SPECS + _SHARED_SPECS:
                if name in ('pm', 'ohr'):
                    continue        # streamed
                t_ = res.tile(shape, dt, tag=name, name=f'sb_{name}')
                nc.sync.dma_start(t_[:], dram_in[name].ap())
                sb[name] = t_

            arenaA = res.tile([128, W, XCOL], BF, tag="arenaA")
            arenaB = res.tile([128, W, XCOL], BF, tag="arenaB")
            nc.vector.memset(arenaA[:, :, FDIM:XCOL], 0.0)
            nc.scalar.memzero(arenaB[:, :, FDIM:XCOL])

            # dinv per node [128, W] f32
            dinv = res.tile([128, W], F32, tag="dinv")
            tmp = wrk.tile([128, W], F32, tag="dtmp")
            nc.vector.tensor_scalar_max(tmp[:], sb['deg_node'][:], 1.0)
            nc.scalar.sqrt(tmp[:], tmp[:])
            nc.vector.reciprocal(dinv[:], tmp[:])

            cc_in = {}
            cc_out = {}
            chunks_of = {1: CHUNKS1, 2: CHUNKS2}
            for l in (1, 2):
                cc_in[l] = [dram.tile([n * 128, XCOL], BF, tag=f"ccin{l}_{c}",
                                      name=f"ccin{l}_{c}")
                            for c, n in enumerate(chunks_of[l])]
                cc_out[l] = dram.tile([ROWS, XCOL], BF,
                                      tag=f"ccout{l}", name=f"ccout{l}")

            def mask_tile(w, t):
                tl = w * (TW + 1) + t
                return sb['mask'][:, tl * 128:(tl + 1) * 128]

            def post_ops(w, hP, l, arena):
                """arena[:, w, 0:FDIM] = [dinv *] ELU(dinv * h + b)."""
                last = l == 3
                bias = sb[f'bias{l}']
                u = post.tile([128, FDIM], F32, tag="u")
                nc.vector.scalar_tensor_tensor(u[:], hP[:], dinv[:, w:w + 1], bias[:],
                                               op0=OP.mult, op1=OP.add)
                v = post.tile([128, FDIM], F32, tag="v")
                nc.vector.tensor_scalar_min(v[:], u[:], 0.0)
                e = post.tile([128, FDIM], F32, tag="e")
                nc.scalar.activation(e[:], v[:], AF.Exp)
                r = post.tile([128, FDIM], F32, tag="r")
                nc.scalar.activation(r[:], u[:], AF.Relu)
                dst = arena[:, w, 0:FDIM]
                if last:
                    nc.vector.scalar_tensor_tensor(dst, e[:], -1.0, r[:],
                                                   op0=OP.add, op1=OP.add)
                else:
                    t2 = post.tile([128, FDIM], F32, tag="t2")
                    nc.vector.scalar_tensor_tensor(t2[:], e[:], -1.0, r[:],
                                                   op0=OP.add, op1=OP.add)
                    nc.scalar.activation(dst, t2[:], AF.Copy, scale=dinv[:, w:w + 1])

            # Per-layer chunk metadata: window -> chunk, chunk row offsets, and
            # the window at which each chunk's AllGather fires (lagged for l=2
            # so the collective's wait doesn't stall the gpsimd gather stream).
            _chunk_meta = {}
            for l in (1, 2):
                chs = chunks_of[l]
                starts = np.cumsum([0] + chs[:-1])
                bases = np.cumsum([0] + [n * NC * 128 for n in chs[:-1]])
                lag = AG_LAG2 if l == 2 else 0
                fire = {}
                for c, n in enumerate(chs):
                    fw = min(starts[c] + n - 1 + (lag if c < len(chs) - 1 else 0), W - 1)
                    fire[fw] = c
                _chunk_meta[l] = (list(starts), list(bases), chs, fire)

            def emit_store_and_ag(l, w, arena):
                """DMA window w rows to its cc_in chunk; fire AG at chunk end."""
                starts, bases, chs, fire = _chunk_meta[l]
                c = int(np.searchsorted(np.cumsum(chs), w, side='right'))
                r0 = (w - starts[c]) * 128
                nc.sync.dma_start(cc_in[l][c][r0:r0 + 128, :], arena[:, w, :])
                if w in fire:
                    cf = fire[w]
                    out0 = bases[cf]
                    outn = chs[cf] * NC * 128
                    nc.gpsimd.collective_compute(
                        "AllGather", OP.bypass, replica_groups=[list(range(NC))],
                        ins=[cc_in[l][cf][:]],
                        outs=[cc_out[l][out0:out0 + outn, :]])

            ctx_pH = tc.tile_pool(name="pH", bufs=2, space="PSUM")
            pH = ctx_pH.__enter__()

            # ================= layer 1 (folded-W1 C-matrix path) =================
            pL1 = ctx_pL1 = tc.tile_pool(name="pL1", bufs=2, space="PSUM")
            pL1 = ctx_pL1.__enter__()
            mask3 = sb['mask'][:].rearrange("p (t c) -> p t c", c=128)
            ohr3 = dram_in['ohr'].ap().rearrange("p (t c) -> p t c", c=CODES)
            # Software-pipelined by one window: dense(w-1) is emitted after
            # cP(w)'s matmuls so the PE never stalls on the A1/A2 evictions.
            prevA = None
            for w in range(W + 1):
                if w < W:
                    t0 = w * (TW + 1)
                    oh = ohpool.tile([128, TW + 1, CODES], BF, tag="oh")
                    nc.sync.dma_start(oh[:], ohr3[:, t0:t0 + TW + 1, :])
                    cP1 = pL1.tile([128, 128], F32, tag="cP1", name="cP1")
                    cP2 = pL1.tile([72, 128], F32, tag="cP2", name="cP2")
                    for t in range(TW + 1):
                        nc.tensor.matmul(cP1[:], lhsT=oh[:, t, 0:128], rhs=mask3[:, t0 + t, :],
                                         start=(t == 0), stop=(t == TW))
                        nc.tensor.matmul(cP2[:], lhsT=oh[:, t, 128:CODES], rhs=mask3[:, t0 + t, :],
                                         start=(t == 0), stop=(t == TW))
                if prevA is not None:
                    A1p, A2p, wp = prevA
                    hP = pH.tile([128, FDIM], F32, tag="h", name="hP1")
                    nc.tensor.matmul(hP[:], lhsT=A1p[:], rhs=sb['TWa'][:], start=True, stop=False)
                    nc.tensor.matmul(hP[:], lhsT=A2p[:], rhs=sb['TWb'][:], start=False, stop=True)
                    post_ops(wp, hP, 1, arenaA)
                    emit_store_and_ag(1, wp, arenaA)
                if w < W:
                    A1 = aggs.tile([128, 128], BF, tag="A1")
                    nc.vector.tensor_copy(A1[:], cP1[:])
                    A2 = aggs.tile([72, 128], BF, tag="A2x", name="A2x")
                    nc.scalar.copy(A2[:], cP2[:])
                    prevA = (A1, A2, w)

            ctx_pL1.__exit__(None, None, None)
            ctx_pA = tc.tile_pool(name="pA", bufs=2, space="PSUM")
            pA = ctx_pA.__enter__()

            # ================= layers 2 and 3 =================
            g1 = pA.tile([128, GSH], F32, tag="g1", name="g1", bufs=1)
            g2 = pA.tile([94, GSH], F32, tag="g2", name="g2", bufs=1)
            for l, arena_prev, arena_next, gname in ((2, arenaA, arenaB, 'gidx2'),
                                                     (3, arenaB, arenaA, 'gidx3')):
                src_full = cc_out[l - 1]
                for w in range(W):
                    aggP1 = pA.tile([128, 128], F32, tag="agg1")
                    aggP2 = pA.tile([94, 128], F32, tag="agg2")
                    fW = fpool.tile([128, TW, XCOL], BF, tag="F")
                    for t in range(TW):
                        nc.gpsimd.indirect_dma_start(
                            out=fW[:, t, :], out_offset=None, in_=src_full[:],
                            in_offset=bass.IndirectOffsetOnAxis(
                                ap=sb[gname][:, w * TW + t:w * TW + t + 1], axis=0))
                    for t in range(TW):
                        fT = fW[:, t, :]
                        nc.tensor.matmul(aggP1[:], lhsT=fT[:, 0:128], rhs=mask_tile(w, t),
                                         start=(t == 0), stop=False)
                        nc.tensor.matmul(aggP2[:], lhsT=fT[:, 128:FDIM], rhs=mask_tile(w, t),
                                         start=(t == 0), stop=False)
                    nc.tensor.matmul(aggP1[:], lhsT=arena_prev[:, w, 0:128],
                                     rhs=mask_tile(w, TW), start=False, stop=True)
                    nc.tensor.matmul(aggP2[:], lhsT=arena_prev[:, w, 128:FDIM],
                                     rhs=mask_tile(w, TW), start=False, stop=True)
                    A1 = aggs.tile([128, 128], BF, tag="A1")
                    nc.vector.tensor_copy(A1[:], aggP1[:])
                    A2 = aggs.tile([94, 128], BF, tag="A2")
                    nc.scalar.copy(A2[:], aggP2[:])
                    hP = pH.tile([128, FDIM], F32, tag="h")
                    nc.tensor.matmul(hP[:], lhsT=A1[:], rhs=sb[f'W{l}a'][:], start=True, stop=False)
                    nc.tensor.matmul(hP[:], lhsT=A2[:], rhs=sb[f'W{l}b'][:], start=False, stop=True)
                    post_ops(w, hP, l, arena_next)
                    if l == 2:
                        emit_store_and_ag(2, w, arena_next)
                    if l == 3:
                        pmT = wrk.tile([128, GSH], BF, tag="pm")
                        nc.sync.dma_start(pmT[:], dram_in['pm'].ap()[:, w * GSH:(w + 1) * GSH])
                        nc.tensor.matmul(g1[:], lhsT=arena_next[:, w, 0:128], rhs=pmT[:],
                                         start=(w == 0), stop=(w == W - 1))
                        nc.tensor.matmul(g2[:], lhsT=arena_next[:, w, 128:FDIM], rhs=pmT[:],
                                         start=(w == 0), stop=(w == W - 1))

            # evict pooled sums, then free psum pools for the head
            g1s = res.tile([128, GSH], F32, tag="g1s")
            nc.scalar.copy(g1s[:], g1[:])
            g2s = res.tile([94, GSH], F32, tag="g2s")
            nc.scalar.copy(g2s[:], g2[:])
            ctx_pA.__exit__(None, None, None)
            ctx_pH.__exit__(None, None, None)

            # ================= head =================
            ctx_pPH = tc.tile_pool(name="pPH", bufs=1, space="PSUM")
            pPH = ctx_pPH.__enter__()
            cibP = pPH.tile([128, GSH], F32, tag="cib", name="cibP")
            nc.tensor.matmul(cibP[:], lhsT=sb['ones_row'][:], rhs=sb['cinv'][:],
                             start=True, stop=True)
            cib = wrk.tile([128, GSH], F32, tag="cibs")
            nc.scalar.copy(cib[:], cibP[:])
            gs1 = res.tile([128, GSH], BF, tag="gs1")
            nc.vector.tensor_tensor(gs1[:], g1s[:], cib[:], op=OP.mult)
            gs2 = res.tile([94, GSH], BF, tag="gs2")
            nc.vector.tensor_tensor(gs2[:], g2s[:], cib[0:94, :], op=OP.mult)

            def elu_head(hp, bias_ap, out_bf):
                u = post.tile(out_bf.shape, F32, tag="u")
                nc.vector.tensor_scalar(u[:], hp[:], bias_ap, None, op0=OP.add)
                v = post.tile(out_bf.shape, F32, tag="v")
                nc.vector.tensor_scalar_min(v[:], u[:], 0.0)
                e = post.tile(out_bf.shape, F32, tag="e")
                nc.scalar.activation(e[:], v[:], AF.Exp)
                r = post.tile(out_bf.shape, F32, tag="r")
                nc.scalar.activation(r[:], u[:], AF.Relu)
                nc.vector.scalar_tensor_tensor(out_bf[:], e[:], -1.0, r[:],
                                               op0=OP.add, op1=OP.add)

            hs1 = []
            for m in range(4):
                hp = pPH.tile([128, GSH], F32, tag="h1p", bufs=2, name="hp")
                nc.tensor.matmul(hp[:], lhsT=sb['Wd1a'][:, 128 * m:128 * (m + 1)],
                                 rhs=gs1[:], start=True, stop=False)
                nc.tensor.matmul(hp[:], lhsT=sb['Wd1b'][:, 128 * m:128 * (m + 1)],
                                 rhs=gs2[:], start=False, stop=True)
                hb = res.tile([128, GSH], BF, tag=f"hs1_{m}")
                elu_head(hp, sb['bd1p'][:, m:m + 1], hb)
                hs1.append(hb)
            h2p = pPH.tile([128, GSH], F32, tag="h2p", name="h2p")
            for m in range(4):
                nc.tensor.matmul(h2p[:], lhsT=sb['Wd2p'][:, 128 * m:128 * (m + 1)],
                                 rhs=hs1[m][:], start=(m == 0), stop=(m == 3))
            hs2 = res.tile([128, GSH], BF, tag="hs2")
            elu_head(h2p, sb['bd2p'][:, 0:1], hs2)
            op_ = pPH.tile([1, GSH], F32, tag="outp", name="op_")
            nc.tensor.matmul(op_[:], lhsT=sb['Wd3p'][:], rhs=hs2[:], start=True, stop=True)
            outS = wrk.tile([1, GSH], F32, tag="outS")
            nc.vector.tensor_scalar(outS[:], op_[:], sb['bd3p'][0:1, 0:1], None, op0=OP.add)
            nc.sync.dma_start(out_t.ap(), outS[:])
            ctx_pPH.__exit__(None, None, None)

    nc.compile()
    _BUILT = (nc, out_t.name)
    return _BUILT


# ---------------- public entry point ----------------

def kernel(elements, oxidations, geometries, angles, edge_index, batch,
           emb_element, emb_ox, emb_geo,
           W1, b1, W2, b2, W3, b3,
           Wd1, bd1, Wd2, bd2, Wd3, bd3):
    global LAST_EXEC_NS, LAST_TRACE_PATH
    inp = dict(elements=elements, oxidations=oxidations, geometries=geometries,
               angles=angles, edge_index=edge_index, batch=batch,
               emb_element=emb_element, emb_ox=emb_ox, emb_geo=emb_geo,
               W1=W1, b1=b1, W2=W2, b2=b2, W3=W3, b3=b3,
               Wd1=Wd1, bd1=bd1, Wd2=Wd2, bd2=bd2, Wd3=Wd3, bd3=bd3)
    pp = _prepare(elements, oxidations, geometries, angles, edge_index, batch)
    wts = _pack_weights(inp)
    nc, out_name = _build()

    in_maps = []
    for k in range(NC):
        c = pp['cores'][k]
        m = {name: c[name] for name, _, _ in _PER_CORE_SPECS}
        for name, _, _ in _SHARED_SPECS:
            m[name] = wts[name]
        in_maps.append(m)

    from concourse import bass_utils
    trace = bool(int(os.environ.get('KERNEL_PROFILE', '0')))
    res = bass_utils.run_bass_kernel_spmd(nc, in_maps, core_ids=list(range(NC)),
                                          trace=trace)
    LAST_EXEC_NS = res.exec_time_ns
    if res.instructions_and_trace is not None:
        LAST_TRACE_PATH = res.instructions_and_trace[1]

    gb = pp['graph_bounds']
    out = np.zeros((N_GRAPHS, 1), f32)
    for k in range(NC):
        ng = pp['cores'][k]['n_graphs']
        out[gb[k]:gb[k + 1], 0] = res.results[k][out_name][0, :ng]
    return out
